# revision 7
# baseline (speedup 1.0000x reference)
"""MLA (Multi-head Latent Attention) fused Bass kernel for 8 TRN2 NeuronCores.

Sharding: core c = 2*b + j handles batch b and a 512-token query slice
(j=0 -> token chunks {0,3} of 256, j=1 -> chunks {1,2}; interleaving
balances causal-attention work). Each core computes K/V for all 1024
tokens of its batch (duplicated within the pair) and emits complete
output rows -> no collectives.

All heavy matmuls run in bf16 (1 cycle/row on the PE) with fp32 PSUM
accumulation. Activations are kept in transposed layout ([feature, token])
so no on-chip transposes are needed anywhere:
  - scores are computed as scoresT[s, tq] = sum_d k[s,d] q[tq,d]
  - softmax skips max-subtraction (scores ~ N(0,1), exp is safe in fp32)
  - the denominator comes from an extra ones-column in V (fits in the
    same M<=128 PV matmuls), normalization is fused into PSUM eviction
  - causal masking is a 0/1 multiply with host-provided mask tiles
  - RoPE uses host-permuted (de-interleaved) rope weights so the rotation
    becomes out = x*C + swap32(x)*S with a PE permutation matmul for swap32
"""

import os
import sys

sys.path.insert(0, "/opt/trn_rl_repo")

import math

import ml_dtypes
import numpy as np

import concourse.bass as bass  # noqa: F401  (import keeps bass registered)
import concourse.mybir as mybir
import concourse.tile as tile
from concourse import bacc
from concourse.bass_utils import run_bass_kernel_spmd

BF = mybir.dt.bfloat16
F32 = mybir.dt.float32
NPBF = ml_dtypes.bfloat16

B, T, C = 4, 1024, 2048
H, DN, DR = 16, 128, 64
D = DN + DR  # 192
QR, KVR = 1536, 512
ROPE_BASE = 10000.0
NCORES = 8
TQ = 512          # query tokens per core
CH = 256          # tq chunk size
NST = 8           # s-tiles of 128
SCALE = 1.0 / math.sqrt(D)

_CACHED_NC = None


def build_nc():
    nc = bacc.Bacc(
        "TRN2",
        target_bir_lowering=False,
        debug=False,
        enable_asserts=True,
        num_devices=NCORES,
    )

    # ---- DRAM parameters (per-core data) ----
    d_xq = nc.dram_tensor("xq", [C, TQ], BF, kind="ExternalInput")
    d_xs = nc.dram_tensor("xs", [C, T], BF, kind="ExternalInput")
    d_wqd = nc.dram_tensor("wqd", [C, QR], BF, kind="ExternalInput")
    d_wkd = nc.dram_tensor("wkd", [C, KVR], BF, kind="ExternalInput")
    d_wqn = nc.dram_tensor("wqn", [QR, H * DN], BF, kind="ExternalInput")
    d_wqr = nc.dram_tensor("wqr", [QR, H * DR], BF, kind="ExternalInput")
    d_wkn = nc.dram_tensor("wkn", [KVR, H * DN], BF, kind="ExternalInput")
    d_wkr = nc.dram_tensor("wkr", [KVR, H * DR], BF, kind="ExternalInput")
    d_wv = nc.dram_tensor("wv", [KVR, H * D], BF, kind="ExternalInput")
    d_wo = nc.dram_tensor("wo", [H * D, C], BF, kind="ExternalInput")
    d_cq = nc.dram_tensor("cq", [128, TQ], BF, kind="ExternalInput")
    d_sq = nc.dram_tensor("sq", [128, TQ], BF, kind="ExternalInput")
    d_ck = nc.dram_tensor("ck", [128, T], BF, kind="ExternalInput")
    d_sk = nc.dram_tensor("sk", [128, T], BF, kind="ExternalInput")
    d_p128 = nc.dram_tensor("p128", [128, 128], BF, kind="ExternalInput")
    d_mA = nc.dram_tensor("mA", [128, 4 * CH], BF, kind="ExternalInput")
    d_mB = nc.dram_tensor("mB", [128, 4 * CH], BF, kind="ExternalInput")
    d_out = nc.dram_tensor("out", [TQ, C], F32, kind="ExternalOutput")

    EXP = mybir.ActivationFunctionType.Exp
    MULT = mybir.AluOpType.mult

    with tile.TileContext(nc) as tc:
        with (
            tc.tile_pool(name="const", bufs=1) as constp,
            tc.tile_pool(name="lat", bufs=1) as latp,
            tc.tile_pool(name="attnout", bufs=1) as attnp,
            tc.tile_pool(name="psmm", bufs=2, space="PSUM") as psmm,
            tc.tile_pool(name="pss", bufs=2, space="PSUM") as pssp,
            tc.tile_pool(name="pspv", bufs=1, space="PSUM") as pspv,
        ):
            # constants
            cq = constp.tile([128, TQ], BF)
            sq = constp.tile([128, TQ], BF)
            ck = constp.tile([128, T], BF)
            sk = constp.tile([128, T], BF)
            p128 = constp.tile([128, 128], BF)
            mA = constp.tile([128, 4 * CH], BF)
            mB = constp.tile([128, 4 * CH], BF)
            nc.sync.dma_start(cq[:], d_cq[:])
            nc.sync.dma_start(sq[:], d_sq[:])
            nc.sync.dma_start(ck[:], d_ck[:])
            nc.sync.dma_start(sk[:], d_sk[:])
            nc.sync.dma_start(p128[:], d_p128[:])
            nc.sync.dma_start(mA[:], d_mA[:])
            nc.sync.dma_start(mB[:], d_mB[:])

            # persistent activations
            q_lat = latp.tile([128, QR // 128, TQ], BF)      # [r%128, rt, tq]
            kv_lat = latp.tile([128, KVR // 128, T], BF)     # [r%128, rt, s]
            attn = attnp.tile([128, 24, TQ], BF)             # [f%128, ft, tq]

            # ---- Phase 1: latents ----
            with tc.tile_pool(name="ph1", bufs=1) as ph1:
                xq_sb = ph1.tile([128, 16, TQ], BF)
                xs_sb = ph1.tile([128, 16, T], BF)
                wqd_sb = ph1.tile([128, 16, QR], BF)
                wkd_sb = ph1.tile([128, 16, KVR], BF)
                nc.sync.dma_start(xq_sb[:], d_xq.rearrange("(k p) n -> p k n", p=128))
                nc.sync.dma_start(xs_sb[:], d_xs.rearrange("(k p) n -> p k n", p=128))
                nc.sync.dma_start(wqd_sb[:], d_wqd.rearrange("(k p) n -> p k n", p=128))
                nc.sync.dma_start(wkd_sb[:], d_wkd.rearrange("(k p) n -> p k n", p=128))

                for rt in range(QR // 128):
                    psq = psmm.tile([128, 512], F32, tag="mm")
                    for kt in range(16):
                        nc.tensor.matmul(
                            psq[:],
                            lhsT=wqd_sb[:, kt, rt * 128:(rt + 1) * 128],
                            rhs=xq_sb[:, kt, :],
                            start=(kt == 0),
                            stop=(kt == 15),
                        )
                    nc.vector.tensor_copy(q_lat[:, rt, :], psq[:])
                for rt in range(KVR // 128):
                    for tch in range(2):
                        psk = psmm.tile([128, 512], F32, tag="mm")
                        for kt in range(16):
                            nc.tensor.matmul(
                                psk[:],
                                lhsT=wkd_sb[:, kt, rt * 128:(rt + 1) * 128],
                                rhs=xs_sb[:, kt, tch * 512:(tch + 1) * 512],
                                start=(kt == 0),
                                stop=(kt == 15),
                            )
                        nc.vector.tensor_copy(kv_lat[:, rt, tch * 512:(tch + 1) * 512], psk[:])

            # ---- Phase 2: per head-pair up-projections + attention ----
            with (
                tc.tile_pool(name="wpair", bufs=2) as wp,
                tc.tile_pool(name="hwork", bufs=2) as hw,
                tc.tile_pool(name="probs", bufs=3) as prp,
                tc.tile_pool(name="small", bufs=2) as smp,
            ):
                for p in range(H // 2):
                    # pair weight slabs
                    wqn_p = wp.tile([128, 12, 256], BF, tag="wqn_p")
                    wqr_p = wp.tile([128, 12, 128], BF, tag="wqr_p")
                    wkn_p = wp.tile([128, 4, 256], BF, tag="wkn_p")
                    wkr_p = wp.tile([128, 4, 128], BF, tag="wkr_p")
                    wv_p = wp.tile([128, 4, 384], BF, tag="wv_p")
                    nc.sync.dma_start(
                        wqn_p[:], d_wqn.rearrange("(k p) n -> p k n", p=128)[:, :, p * 256:(p + 1) * 256]
                    )
                    nc.sync.dma_start(
                        wqr_p[:], d_wqr.rearrange("(k p) n -> p k n", p=128)[:, :, p * 128:(p + 1) * 128]
                    )
                    nc.sync.dma_start(
                        wkn_p[:], d_wkn.rearrange("(k p) n -> p k n", p=128)[:, :, p * 256:(p + 1) * 256]
                    )
                    nc.sync.dma_start(
                        wkr_p[:], d_wkr.rearrange("(k p) n -> p k n", p=128)[:, :, p * 128:(p + 1) * 128]
                    )
                    nc.sync.dma_start(
                        wv_p[:], d_wv.rearrange("(k p) n -> p k n", p=128)[:, :, p * 384:(p + 1) * 384]
                    )

                    # --- up-projections ---
                    qc = []
                    kc = []
                    for w in range(2):
                        psq2 = psmm.tile([128, 512], F32, tag="mm")
                        for kt in range(12):
                            nc.tensor.matmul(
                                psq2[:],
                                lhsT=wqn_p[:, kt, w * 128:(w + 1) * 128],
                                rhs=q_lat[:, kt, :],
                                start=(kt == 0),
                                stop=(kt == 11),
                            )
                        qc_w = hw.tile([128, TQ], BF, tag=f"qc{w}")
                        nc.vector.tensor_copy(qc_w[:], psq2[:])
                        qc.append(qc_w)

                        kc_w = hw.tile([128, T], BF, tag=f"kc{w}")
                        for tch in range(2):
                            psk2 = psmm.tile([128, 512], F32, tag="mm")
                            for kt in range(4):
                                nc.tensor.matmul(
                                    psk2[:],
                                    lhsT=wkn_p[:, kt, w * 128:(w + 1) * 128],
                                    rhs=kv_lat[:, kt, tch * 512:(tch + 1) * 512],
                                    start=(kt == 0),
                                    stop=(kt == 3),
                                )
                            nc.vector.tensor_copy(kc_w[:, tch * 512:(tch + 1) * 512], psk2[:])
                        kc.append(kc_w)

                    # --- rope: q (both heads of pair share the [128, TQ] tile) ---
                    psr = psmm.tile([128, 512], F32, tag="mm")
                    for kt in range(12):
                        nc.tensor.matmul(
                            psr[:],
                            lhsT=wqr_p[:, kt, :],
                            rhs=q_lat[:, kt, :],
                            start=(kt == 0),
                            stop=(kt == 11),
                        )
                    qro_raw = hw.tile([128, TQ], BF, tag="qro_raw")
                    nc.vector.tensor_copy(qro_raw[:], psr[:])
                    psw = psmm.tile([128, 512], F32, tag="mm")
                    nc.tensor.matmul(psw[:], lhsT=p128[:], rhs=qro_raw[:], start=True, stop=True)
                    qro = hw.tile([128, TQ], BF, tag="qro")
                    qtmp = hw.tile([128, TQ], BF, tag="qtmp")
                    nc.vector.tensor_tensor(qtmp[:], psw[:], sq[:], MULT)
                    nc.vector.tensor_tensor(qro[:], qro_raw[:], cq[:], MULT)
                    nc.vector.tensor_add(qro[:], qro[:], qtmp[:])

                    # --- rope: k ---
                    kro_raw = hw.tile([128, T], BF, tag="kro_raw")
                    for tch in range(2):
                        psr2 = psmm.tile([128, 512], F32, tag="mm")
                        for kt in range(4):
                            nc.tensor.matmul(
                                psr2[:],
                                lhsT=wkr_p[:, kt, :],
                                rhs=kv_lat[:, kt, tch * 512:(tch + 1) * 512],
                                start=(kt == 0),
                                stop=(kt == 3),
                            )
                        nc.vector.tensor_copy(kro_raw[:, tch * 512:(tch + 1) * 512], psr2[:])
                    kro = hw.tile([128, T], BF, tag="kro")
                    ktmp = hw.tile([128, T], BF, tag="ktmp")
                    for tch in range(2):
                        sl = slice(tch * 512, (tch + 1) * 512)
                        psw2 = psmm.tile([128, 512], F32, tag="mm")
                        nc.tensor.matmul(psw2[:], lhsT=p128[:], rhs=kro_raw[:, sl], start=True, stop=True)
                        nc.vector.tensor_tensor(ktmp[:, sl], psw2[:], sk[:, sl], MULT)
                    nc.vector.tensor_tensor(kro[:], kro_raw[:], ck[:], MULT)
                    nc.vector.tensor_add(kro[:], kro[:], ktmp[:])

                    # --- v (both heads): cols [he d0:192 | ones | ho d0:192 | ones] ---
                    v_pr = hw.tile([128, 8, 386], BF, tag="v_pr")
                    for st in range(NST):
                        psv = psmm.tile([128, 384], F32, tag="mm")
                        for kt in range(4):
                            nc.tensor.matmul(
                                psv[:],
                                lhsT=kv_lat[:, kt, st * 128:(st + 1) * 128],
                                rhs=wv_p[:, kt, :],
                                start=(kt == 0),
                                stop=(kt == 3),
                            )
                        nc.vector.tensor_copy(v_pr[:, st, 0:192], psv[:, 0:192])
                        nc.vector.tensor_copy(v_pr[:, st, 193:385], psv[:, 192:384])
                    nc.vector.memset(v_pr[:, :, 192:193], 1.0)
                    nc.vector.memset(v_pr[:, :, 385:386], 1.0)

                    # --- attention for both heads of the pair ---
                    for w in range(2):
                        for chn in range(2):
                            nst = 4 if chn == 0 else NST
                            csl = slice(chn * CH, (chn + 1) * CH)
                            psA = pspv.tile([128, 256], F32, tag="psA")
                            psB = pspv.tile([128, 256], F32, tag="psB")
                            psD = (
                                pspv.tile([128, 256], F32, tag="psD", name="psD")
                                if w == 1
                                else None
                            )
                            for st in range(nst):
                                pss = pssp.tile([128, 256], F32, tag="pss")
                                nc.tensor.matmul(
                                    pss[:],
                                    lhsT=kc[w][:, st * 128:(st + 1) * 128],
                                    rhs=qc[w][:, csl],
                                    start=True,
                                    stop=False,
                                )
                                nc.tensor.matmul(
                                    pss[:],
                                    lhsT=kro[w * 64:(w + 1) * 64, st * 128:(st + 1) * 128],
                                    rhs=qro[w * 64:(w + 1) * 64, csl],
                                    start=False,
                                    stop=True,
                                )
                                pr = prp.tile([128, 256], BF, tag="pr")
                                nc.scalar.activation(pr[:], pss[:], EXP, scale=SCALE)
                                if chn == 0:
                                    nc.vector.tensor_tensor(
                                        pr[:], pr[:], mA[:, st * CH:(st + 1) * CH], MULT
                                    )
                                elif st >= 4:
                                    nc.vector.tensor_tensor(
                                        pr[:], pr[:], mB[:, (st - 4) * CH:(st - 3) * CH], MULT
                                    )
                                first = st == 0
                                last = st == nst - 1
                                if w == 0:
                                    nc.tensor.matmul(
                                        psA[0:128, :], lhsT=v_pr[:, st, 0:128], rhs=pr[:],
                                        start=first, stop=last,
                                    )
                                    nc.tensor.matmul(
                                        psB[0:65, :], lhsT=v_pr[:, st, 128:193], rhs=pr[:],
                                        start=first, stop=last,
                                    )
                                else:
                                    nc.tensor.matmul(
                                        psA[64:128, :], lhsT=v_pr[:, st, 193:257], rhs=pr[:],
                                        start=first, stop=last,
                                    )
                                    nc.tensor.matmul(
                                        psB[0:128, :], lhsT=v_pr[:, st, 257:385], rhs=pr[:],
                                        start=first, stop=last,
                                    )
                                    nc.tensor.matmul(
                                        psD[0:1, :], lhsT=v_pr[:, st, 385:386], rhs=pr[:],
                                        start=first, stop=last,
                                    )
                            # normalize + evict into attn ([f%128, ft, tq])
                            r_sb = smp.tile([1, 256], F32, tag="r_sb")
                            denom = psB[64:65, :] if w == 0 else psD[0:1, :]
                            nc.vector.reciprocal(r_sb[:], denom)
                            Rb = smp.tile([128, 256], F32, tag="Rb")
                            nc.gpsimd.partition_broadcast(Rb[:], r_sb[:])
                            k0 = 3 * p + w  # h even: blocks (3p, 3p+1); odd: (3p+1, 3p+2)
                            if w == 0:
                                nc.vector.tensor_tensor(
                                    attn[0:128, k0, csl], psA[0:128, :], Rb[0:128, :], MULT
                                )
                                nc.vector.tensor_tensor(
                                    attn[0:64, k0 + 1, csl], psB[0:64, :], Rb[0:64, :], MULT
                                )
                            else:
                                nc.vector.tensor_tensor(
                                    attn[64:128, k0, csl], psA[64:128, :], Rb[64:128, :], MULT
                                )
                                nc.vector.tensor_tensor(
                                    attn[0:128, k0 + 1, csl], psB[0:128, :], Rb[0:128, :], MULT
                                )

            # ---- Phase 3: output projection ----
            with (
                tc.tile_pool(name="wo", bufs=2) as wop,
                tc.tile_pool(name="osb", bufs=3) as osbp,
            ):
                for cch in range(4):
                    wo_sb = wop.tile([128, 24, 512], BF, tag="wo_sb")
                    nc.sync.dma_start(
                        wo_sb[:],
                        d_wo.rearrange("(k p) n -> p k n", p=128)[:, :, cch * 512:(cch + 1) * 512],
                    )
                    for tt in range(4):
                        pso = psmm.tile([128, 512], F32, tag="mm")
                        for kt in range(24):
                            nc.tensor.matmul(
                                pso[:],
                                lhsT=attn[:, kt, tt * 128:(tt + 1) * 128],
                                rhs=wo_sb[:, kt, :],
                                start=(kt == 0),
                                stop=(kt == 23),
                            )
                        osb = osbp.tile([128, 512], F32, tag="osb")
                        nc.vector.tensor_copy(osb[:], pso[:])
                        nc.sync.dma_start(
                            d_out[tt * 128:(tt + 1) * 128, cch * 512:(cch + 1) * 512], osb[:]
                        )

    nc.compile()
    return nc


# ---------------- host-side preparation ----------------

def _tq_cols(j):
    if j == 0:
        return np.concatenate([np.arange(0, 256), np.arange(768, 1024)])
    return np.arange(256, 768)


def _rope_tables():
    inv = ROPE_BASE ** (-np.arange(0, DR, 2, dtype=np.float64) / DR)  # [32]
    t = np.arange(T, dtype=np.float64)
    ang = np.outer(t, inv)  # [T, 32]
    cosT = np.cos(ang).T.astype(np.float32)  # [32, T]
    sinT = np.sin(ang).T.astype(np.float32)
    Ck = np.tile(cosT, (4, 1))  # [128, T]
    Sk = np.concatenate([-sinT, sinT, -sinT, sinT], axis=0)  # [128, T]
    return Ck, Sk


def _deinterleave_cols(w):
    # [R, H*DR] with head cols (2i, 2i+1 interleaved) -> per head [x1(32) | x2(32)]
    r = w.shape[0]
    return w.reshape(r, H, DR // 2, 2).transpose(0, 1, 3, 2).reshape(r, H * DR)


def _swap_matrix():
    P = np.zeros((128, 128), np.float32)
    for blk in range(2):
        for i in range(32):
            P[blk * 64 + i, blk * 64 + 32 + i] = 1.0
            P[blk * 64 + 32 + i, blk * 64 + i] = 1.0
    return P


def _masks(j):
    cA, cB = ((0, 3) if j == 0 else (1, 2))
    s = np.arange(T)[:, None]
    t = np.arange(T)[None, :]
    valid = (s <= t).astype(np.float32)  # [s, t]
    # chunk A: s-tiles 0..3 (s rows 0:512), cols = chunk cA
    mAv = valid[0:512, cA * CH:(cA + 1) * CH]  # [512, 256]
    mA = mAv.reshape(4, 128, CH).transpose(1, 0, 2).reshape(128, 4 * CH)
    # chunk B: s-tiles 4..7 (s rows 512:1024), cols = chunk cB
    mBv = valid[512:1024, cB * CH:(cB + 1) * CH]
    mB = mBv.reshape(4, 128, CH).transpose(1, 0, 2).reshape(128, 4 * CH)
    return mA, mB


def make_in_maps(inputs):
    x = np.asarray(inputs["x"], np.float32)
    wqd = np.asarray(inputs["wq_down"], np.float32).astype(NPBF)
    wkd = np.asarray(inputs["wkv_down"], np.float32).astype(NPBF)
    wqn = np.asarray(inputs["wq_nope"], np.float32).astype(NPBF)
    wqr = _deinterleave_cols(np.asarray(inputs["wq_rope"], np.float32)).astype(NPBF)
    wkn = np.asarray(inputs["wk_nope"], np.float32).astype(NPBF)
    wkr = _deinterleave_cols(np.asarray(inputs["wk_rope"], np.float32)).astype(NPBF)
    wv = np.asarray(inputs["wv_up"], np.float32).astype(NPBF)
    wo = np.asarray(inputs["wo"], np.float32).astype(NPBF)

    Ck, Sk = _rope_tables()
    P = _swap_matrix().astype(NPBF)

    in_maps = []
    for c in range(NCORES):
        b, j = c // 2, c % 2
        cols = _tq_cols(j)
        xT = np.ascontiguousarray(x[b].T).astype(NPBF)  # [C, T]
        xq = np.ascontiguousarray(xT[:, cols])
        mA, mB = _masks(j)
        in_maps.append(
            {
                "xq": xq,
                "xs": xT,
                "wqd": wqd,
                "wkd": wkd,
                "wqn": wqn,
                "wqr": wqr,
                "wkn": wkn,
                "wkr": wkr,
                "wv": wv,
                "wo": wo,
                "cq": np.ascontiguousarray(Ck[:, cols]).astype(NPBF),
                "sq": np.ascontiguousarray(Sk[:, cols]).astype(NPBF),
                "ck": Ck.astype(NPBF),
                "sk": Sk.astype(NPBF),
                "p128": P,
                "mA": mA.astype(NPBF),
                "mB": mB.astype(NPBF),
            }
        )
    return in_maps


def assemble_output(results):
    out = np.empty((B, T, C), np.float32)
    for c in range(NCORES):
        b, j = c // 2, c % 2
        out[b, _tq_cols(j), :] = results[c]["out"]
    return out


def kernel(**inputs):
    global _CACHED_NC
    if _CACHED_NC is None:
        _CACHED_NC = build_nc()
    in_maps = make_in_maps(inputs)
    trace = bool(int(os.environ.get("MLA_TRACE", "0")))
    try:
        res = run_bass_kernel_spmd(
            _CACHED_NC, in_maps, core_ids=list(range(NCORES)), trace=trace
        )
    except ModuleNotFoundError:
        # no NTFF profiling hook in this environment -> run untraced
        res = run_bass_kernel_spmd(
            _CACHED_NC, in_maps, core_ids=list(range(NCORES)), trace=False
        )
    out = assemble_output(res.results)
    if trace:
        kernel.last_exec_time_ns = res.exec_time_ns
        kernel.last_results = res
    return out
